# revision 2
# baseline (speedup 1.0000x reference)
"""LIF (leaky integrate-and-fire) scan kernel for Trainium2, 8 NeuronCores.

Reference semantics (fp32, T=8 innermost axis):
    mem = 0
    for t in range(T):
        mem = mem * 0.5 + x[..., t]
        s[..., t] = (mem >= 1.0)
        mem = mem * (1.0 - s[..., t])

Sharding: data-parallel over the leading dim (64 -> 8 per core).

Memory-roofline design: input must stream 32 MiB/core of fp32, but the
output is binary, so it leaves the device as uint8 (8 MiB/core instead of
32 MiB).  The host maps spikes back with (y == 1) -> f32, which is immune
to whether the device's f32->u8 conversion saturates or wraps.

Per-core layout is chunk-major/t-minor: x[p, (c*T + t)*CH + n] so every
strip the device touches is contiguous.  The per-timestep ops are split
across three engines so no engine exceeds the input-DMA time:
    A (DVE):    m = (r mult 0.5) add x_t          scalar_tensor_tensor
    B (Act):    y_t = Sign(m - 1) -> uint8        activation (sat to {0,1})
    C (GpSimd): r = (m is_lt 1) mult m            scalar_tensor_tensor
A is skipped at t=0 (mem0=0 -> m is just x_0) and C at t=T-1 (dead value).

Input strips ride the otherwise-idle qSP hardware DGE queue (nc.sync);
output chunks ride qAct (nc.scalar).  Chunks are processed in groups of 3
with t-outer/chunk-inner issue order so the in-order engines always have
an independent chunk to work on while a chunk waits on the cross-engine
recurrence; the 27-deep input ring lets the DMA queue run a full group
ahead of compute.
"""

import numpy as np

import concourse.bass as bass
import concourse.tile as tile
from concourse import bacc, mybir
from concourse.bass_utils import run_bass_kernel_spmd

P = 128           # SBUF partitions
T = 8             # timesteps (innermost axis of the original input)
NPB = 8192        # neurons per partition per core: 8*128*32*32 / 128
CH = 1024         # neurons per chunk (per partition)
NCH = NPB // CH   # 8 chunks
GROUPS = [[0, 1, 2], [3, 4, 5], [6, 7]]

THRESH = 1.0
DECAY = 0.5
F32 = mybir.dt.float32
U8 = mybir.dt.uint8
N_CORES = 8

Alu = mybir.AluOpType
Act = mybir.ActivationFunctionType

# Spike op placement: "act" uses the Activation engine (Sign(m-1) saturated
# to u8); "dve" uses a 2x-mode tensor_scalar is_ge on the Vector engine.
B_ENGINE = "act"


def _build() -> bass.Bass:
    nc = bacc.Bacc("TRN2", target_bir_lowering=False, debug=False)
    x = nc.dram_tensor("x", [P, NCH * T * CH], F32, kind="ExternalInput").ap()
    y = nc.dram_tensor("y", [P, NCH * T * CH], U8, kind="ExternalOutput").ap()

    def spike(out_ap, m_ap):
        if B_ENGINE == "act":
            nc.scalar.activation(out_ap, m_ap, Act.Sign, bias=-float(THRESH))
        else:
            nc.vector.tensor_scalar(
                out_ap, m_ap, THRESH, None, Alu.is_ge, Alu.bypass
            )

    with tile.TileContext(nc) as tc:
        with (
            tc.tile_pool(name="xs", bufs=27) as xpool,
            tc.tile_pool(name="ys", bufs=5) as ypool,
            tc.tile_pool(name="state", bufs=4) as spool,
        ):
            for chunks in GROUPS:
                # Input strips: t-outer / c-inner, all on the qSP HW queue.
                xs = {c: [None] * T for c in chunks}
                for t in range(T):
                    for c in chunks:
                        st = xpool.tile([P, CH], F32, tag="x", name=f"x{c}_{t}")
                        nc.sync.dma_start(
                            st[:], x[:, (c * T + t) * CH : (c * T + t + 1) * CH]
                        )
                        xs[c][t] = st

                yt, m, r = {}, {}, {}
                for c in chunks:
                    yt[c] = ypool.tile([P, T * CH], U8, tag="y", name=f"y{c}")
                    m[c] = spool.tile([P, CH], F32, tag="m", name=f"m{c}")
                    r[c] = spool.tile([P, CH], F32, tag="r", name=f"r{c}")

                # t = 0: mem0 = 0 so m == x_0; spike and reset read the strip.
                for c in chunks:
                    spike(yt[c][:, 0:CH], xs[c][0][:])
                for c in chunks:
                    nc.gpsimd.scalar_tensor_tensor(
                        r[c][:], xs[c][0][:], THRESH, xs[c][0][:],
                        Alu.is_lt, Alu.mult,
                    )

                for t in range(1, T):
                    for c in chunks:
                        nc.vector.scalar_tensor_tensor(
                            m[c][:], r[c][:], DECAY, xs[c][t][:],
                            Alu.mult, Alu.add,
                        )
                    for c in chunks:
                        spike(yt[c][:, t * CH : (t + 1) * CH], m[c][:])
                    if t < T - 1:
                        for c in chunks:
                            nc.gpsimd.scalar_tensor_tensor(
                                r[c][:], m[c][:], THRESH, m[c][:],
                                Alu.is_lt, Alu.mult,
                            )

                # Output: one contiguous u8 chunk per c on the qAct HW queue.
                for c in chunks:
                    nc.scalar.dma_start(
                        y[:, c * T * CH : (c + 1) * T * CH], yt[c][:]
                    )
    nc.compile()
    return nc


_NC_CACHE: bass.Bass | None = None


def _get_nc() -> bass.Bass:
    global _NC_CACHE
    if _NC_CACHE is None:
        _NC_CACHE = _build()
    return _NC_CACHE


def _run(X: np.ndarray, **spmd_kwargs):
    assert X.shape == (64, 128, 32, 32, 8), X.shape
    X = np.ascontiguousarray(X, dtype=np.float32)
    per_core = 64 // N_CORES
    # [core, p, nch, ch, t] -> chunk-major t-minor [core, p, nch, t, ch]
    Xt = np.ascontiguousarray(
        X.reshape(N_CORES, P, NCH, CH, T).transpose(0, 1, 2, 4, 3)
    )
    in_maps = [{"x": Xt[i].reshape(P, NCH * T * CH)} for i in range(N_CORES)]
    res = run_bass_kernel_spmd(
        _get_nc(), in_maps, core_ids=list(range(N_CORES)), **spmd_kwargs
    )
    out = np.empty_like(X)
    for i, r in enumerate(res.results):
        s = r["y"].reshape(P, NCH, T, CH).transpose(0, 1, 3, 2)
        out[i * per_core : (i + 1) * per_core] = (
            (s == 1).astype(np.float32).reshape(per_core, 128, 32, 32, 8)
        )
    return out, res


def kernel(X: np.ndarray) -> np.ndarray:
    out, _ = _run(X)
    return out
